# revision 45
# baseline (speedup 1.0000x reference)
"""SAGEConv (mean aggregation) GNN message passing on 8 Trainium2 NeuronCores.

    out_i = lin_l(mean_{j:(j->i) in E} x_j) + lin_r(x_i)

Strategy (graph partitioning by destination node):
  - Host: shard dst nodes across 8 cores (2500 each). Per core, sort its
    incoming edges by dst, group into 20 tiles of 128 dst nodes. Per tile,
    edges are sorted by src (ascending HBM addresses improve DMA locality)
    and padded to nb_t blocks of 128 edges, where nb_t is the max block
    count across cores for tile slot t (keeps the program SPMD-identical).
  - Device (per core), per dst tile:
      * one large SWDGE dma_gather pulls all the tile's edge source rows
        (bf16, 256B each) from the replicated feature table in HBM into
        SBUF (amortizes the ~1us per-call SWDGE fixed cost).
      * DVE builds the scaled one-hot S[e,(b,d)] = (dslot_e,b == d) via
        is_equal against a materialized iota pattern; PE accumulates
        aggT[f,d] += sum_e M[e,f]*S[e,d] over the tile's blocks in PSUM.
      * aggT is scaled by 1/cnt during the PSUM->SBUF move (DVE mult).
      * PE applies W_l/W_r in bf16 (+ bias via a rank-1 matmul),
        accumulated in PSUM; scalar engine copies to SBUF; HWDGE DMA
        writes the 128-row output tile.
  - Host: concatenate the 8 per-core [2500, 128] outputs.
"""

import contextlib
import ctypes
import sys
import types

import ml_dtypes
import numpy as np

# ---------------------------------------------------------------------------
# NTFF profiling hook (lets run_bass_kernel_spmd(trace=True) work under axon;
# harmless if tracing is never requested).
# ---------------------------------------------------------------------------
_AXON_SO = "/opt/axon/libaxon_pjrt.so"


def _install_axon_ntff_hook():
    if "antenv.axon_hooks" in sys.modules:
        return
    try:
        lib = ctypes.CDLL(_AXON_SO)
        if not hasattr(lib, "axon_start_nrt_profile"):
            raise OSError("no profile symbols")
        lib.axon_start_nrt_profile.argtypes = [
            ctypes.POINTER(ctypes.c_int64),
            ctypes.c_size_t,
        ]
        lib.axon_start_nrt_profile.restype = ctypes.c_int64
        lib.axon_stop_nrt_profile.argtypes = [ctypes.c_char_p]
        lib.axon_stop_nrt_profile.restype = ctypes.c_int64

        @contextlib.contextmanager
        def _hook(output_dir, device_ids):
            import jax

            jax.devices()
            if device_ids:
                ids = (ctypes.c_int64 * len(device_ids))(*device_ids)
                rc = lib.axon_start_nrt_profile(ids, len(device_ids))
            else:
                rc = lib.axon_start_nrt_profile(None, 0)
            if rc != 0:
                raise RuntimeError(f"axon_start_nrt_profile rc={rc}")
            try:
                yield
            finally:
                n = lib.axon_stop_nrt_profile(str(output_dir).encode())
                print(f"ntff profile: {n} file(s) -> {output_dir}", file=sys.stderr)

        hook = _hook
    except OSError:
        hook = None

    mod = types.ModuleType("antenv.axon_hooks")
    mod._hook = hook
    mod.get_axon_ntff_profile_hook = lambda: mod._hook
    mod.set_axon_ntff_profile_hook = lambda h: setattr(mod, "_hook", h)
    sys.modules["antenv.axon_hooks"] = mod
    try:
        import antenv

        antenv.axon_hooks = mod
    except ImportError:
        pass


_install_axon_ntff_hook()

import concourse.bacc as bacc  # noqa: E402
import concourse.mybir as mybir  # noqa: E402
import concourse.tile as tile  # noqa: E402
from concourse.bass_utils import run_bass_kernel_spmd  # noqa: E402

# Problem shape (hardcoded per spec).
N_NODES = 20000
N_EDGES = 640000
HIDDEN = 128
N_CORES = 8
NODES_PER_CORE = N_NODES // N_CORES  # 2500
P = 128
N_TILES = -(-NODES_PER_CORE // P)  # 20 dst tiles per core (last has 68 rows)
LAST_ROWS = NODES_PER_CORE - (N_TILES - 1) * P  # 68

BF16 = ml_dtypes.bfloat16

_compiled_cache = {}


def _build_bass(nb_sched: tuple[int, ...]):
    """Per-core Bass program; nb_sched[t] = edge-block count for dst tile t."""
    nb_tot = sum(nb_sched)
    nc = bacc.Bacc(
        target_bir_lowering=False,
        num_swdge_queues=4,
        dynamic_dma_scratch_size=65536,
    )
    dt = mybir.dt

    feat = nc.dram_tensor("feat", [N_NODES, HIDDEN], dt.bfloat16, kind="ExternalInput")
    idx_all = nc.dram_tensor("idx_all", [P, nb_tot * 8], dt.int16, kind="ExternalInput")
    dslot = nc.dram_tensor("dslot", [P, nb_tot], dt.bfloat16, kind="ExternalInput")
    invb = nc.dram_tensor("invb", [P, N_TILES * P], dt.bfloat16, kind="ExternalInput")
    xt = nc.dram_tensor("xt", [P, N_TILES * P], dt.bfloat16, kind="ExternalInput")
    wlt = nc.dram_tensor("wlt", [P, HIDDEN], dt.bfloat16, kind="ExternalInput")
    wrt = nc.dram_tensor("wrt", [P, HIDDEN], dt.bfloat16, kind="ExternalInput")
    bias1 = nc.dram_tensor("bias1", [1, HIDDEN], dt.bfloat16, kind="ExternalInput")
    ones1 = nc.dram_tensor("ones1", [1, HIDDEN], dt.bfloat16, kind="ExternalInput")
    iota = nc.dram_tensor("iota", [P, P], dt.bfloat16, kind="ExternalInput")
    out = nc.dram_tensor("out", [NODES_PER_CORE, HIDDEN], dt.bfloat16, kind="ExternalOutput")

    with tile.TileContext(nc) as tc:
        with (
            tc.tile_pool(name="const", bufs=1) as cpool,
            tc.tile_pool(name="meta", bufs=1) as mpool,
            tc.tile_pool(name="gath", bufs=5) as gpool,
            tc.tile_pool(name="sel", bufs=5) as spool,
            tc.tile_pool(name="aggs", bufs=2) as apool,
            tc.tile_pool(name="outs", bufs=2) as opool,
            tc.tile_pool(name="pagg", bufs=4, space="PSUM") as pagg_pool,
            tc.tile_pool(name="pout", bufs=2, space="PSUM") as pout_pool,
        ):
            # One-time loads.
            iota_t = cpool.tile([P, P], dt.bfloat16)
            wlt_t = cpool.tile([P, HIDDEN], dt.bfloat16, tag="wlt")
            wrt_t = cpool.tile([P, HIDDEN], dt.bfloat16, tag="wrt")
            bias1_t = cpool.tile([1, HIDDEN], dt.bfloat16, tag="bias1")
            ones1_t = cpool.tile([1, HIDDEN], dt.bfloat16, tag="ones1")
            xt_t = cpool.tile([P, N_TILES * P], dt.bfloat16, tag="xt")
            idx_t = mpool.tile([P, nb_tot * 8], dt.int16, tag="idx")
            dslot_t = mpool.tile([P, nb_tot], dt.bfloat16, tag="dslot")
            invb_t = mpool.tile([P, N_TILES * P], dt.bfloat16, tag="invb")
            # Two-chunk idx load: the first tiles' indices land quickly so
            # tile 0's gathers start early; the bulk loads behind them.
            c1 = 8 * sum(nb_sched[:3])
            nc.sync.dma_start(idx_t[:, :c1], idx_all[:, :c1])
            nc.sync.dma_start(dslot_t[:], dslot[:])
            nc.sync.dma_start(idx_t[:, c1:], idx_all[:, c1:])
            nc.sync.dma_start(iota_t[:], iota[:])
            nc.sync.dma_start(wlt_t[:], wlt[:])
            nc.sync.dma_start(wrt_t[:], wrt[:])
            nc.sync.dma_start(bias1_t[:], bias1[:])
            nc.sync.dma_start(ones1_t[:], ones1[:])
            nc.sync.dma_start(xt_t[:], xt[:])
            nc.sync.dma_start(invb_t[:], invb[:])

            off = 0
            _qn = [0]
            for t in range(N_TILES):
                nb = nb_sched[t]
                g = gpool.tile([P, nb, HIDDEN], dt.bfloat16, tag="g")
                # Split each tile's gather across the 4 SWDGE queues: the four
                # Q7 pairs generate in parallel and the SDMA drains (the
                # per-tile wall) overlap.
                splits = [(j * nb) // 4 for j in range(5)]
                for j in range(4):
                    lo, hi = splits[j], splits[j + 1]
                    if hi == lo:
                        continue
                    nc.gpsimd.dma_gather(
                        g[:, lo:hi, :],
                        feat[:, :],
                        idx_t[:, (off + lo) * 8 : (off + hi) * 8],
                        num_idxs=(hi - lo) * P,
                        num_idxs_reg=(hi - lo) * P,
                        elem_size=HIDDEN,
                        queue_num=j,
                        single_packet=False,
                    )
                # One-hot S[e,(b,d)] = (dslot[e,b] == d), built in two steps so
                # the DVE compare hits the 2x perf tier (needs stride-1 last
                # dims on all operands): scalar engine replicates dslot along
                # d in-place, then DVE compares against the iota pattern.
                s = spool.tile([P, nb, P], dt.bfloat16, tag="s")
                nc.scalar.copy(
                    s[:],
                    dslot_t[:, off : off + nb][:, :, None].to_broadcast([P, nb, P]),
                )
                nc.vector.tensor_tensor(
                    s[:],
                    iota_t[:, None, :].to_broadcast([P, nb, P]),
                    s[:],
                    op=mybir.AluOpType.is_equal,
                )
                pa = pagg_pool.tile([P, P], dt.float32, tag="pa")
                for b in range(nb):
                    nc.tensor.matmul(
                        pa[:],
                        lhsT=g[:, b, :],
                        rhs=s[:, b, :],
                        start=(b == 0),
                        stop=(b == nb - 1),
                    )
                # mean: aggT = psum * (1/cnt[d]) during PSUM -> SBUF move.
                at = apool.tile([P, P], dt.bfloat16, tag="at")
                nc.vector.tensor_tensor(
                    at[:], pa[:], invb_t[:, t * P : (t + 1) * P], op=mybir.AluOpType.mult
                )
                po = pout_pool.tile([P, P], dt.float32, tag="po")
                nc.tensor.matmul(po[:], lhsT=ones1_t[:], rhs=bias1_t[:], start=True, stop=False)
                nc.tensor.matmul(po[:], lhsT=at[:], rhs=wlt_t[:], start=False, stop=False)
                nc.tensor.matmul(
                    po[:],
                    lhsT=xt_t[:, t * P : (t + 1) * P],
                    rhs=wrt_t[:],
                    start=False,
                    stop=True,
                )
                ob = opool.tile([P, P], dt.bfloat16, tag="ob")
                nc.scalar.copy(ob[:], po[:])
                rows = LAST_ROWS if t == N_TILES - 1 else P
                nc.sync.dma_start(out[t * P : t * P + rows, :], ob[:rows, :])
                off += nb
    nc.compile()
    return nc


def _prepare_shards(features, edge_index, W_l, b_l, W_r):
    """Host-side graph partitioning -> per-core input maps + nb schedule."""
    src = np.asarray(edge_index[0], dtype=np.int64)
    dst = np.asarray(edge_index[1], dtype=np.int64)
    feats = np.asarray(features, dtype=np.float32)

    cnt = np.bincount(dst, minlength=N_NODES).astype(np.float32)
    inv = (1.0 / np.maximum(cnt, 1.0)).astype(np.float32)

    # Sort edges by (dst tile, src): group per (core, tile), ascending src
    # within each group for better HBM read locality during the gather.
    core_of_all = dst // NODES_PER_CORE
    off_all = dst - core_of_all * NODES_PER_CORE
    tile_of_all = off_all // P
    flat_ct_all = core_of_all * N_TILES + tile_of_all
    order = np.lexsort((src, flat_ct_all))
    src_s = src[order]
    dst_s = dst[order]
    flat_ct = flat_ct_all[order]
    slot_of = (dst_s % NODES_PER_CORE) % P

    # Edge counts per (core, tile); shared per-tile block schedule (max
    # across cores so the SPMD program is identical on every core).
    n_ct = N_CORES * N_TILES
    ct_cnt = np.bincount(flat_ct, minlength=n_ct).reshape(N_CORES, N_TILES)
    nb_sched = tuple(int(x) for x in np.maximum(-(-ct_cnt.max(axis=0) // P), 1))
    nb_tot = sum(nb_sched)
    tile_off = np.concatenate(([0], np.cumsum(nb_sched))).astype(np.int64)

    # Scatter each group's edges into its padded [nb_t*128] slot range.
    # Each tile's gather is issued as 4 splits (block ranges [(j*nb)//4]);
    # spread the group's real edges across the splits proportionally so
    # padding (-1 indices, dropped by the Q7) sits at each split's tail.
    starts = np.zeros(n_ct + 1, dtype=np.int64)
    np.cumsum(ct_cnt.reshape(-1), out=starts[1:])
    pos_in_group = np.arange(dst_s.shape[0]) - starts[flat_ct]
    core_idx = flat_ct // N_TILES
    tile_idx = flat_ct % N_TILES

    # flat position within the per-core padded edge array of length nb_tot*128
    flat_pos = (core_idx * nb_tot + tile_off[tile_idx]) * P + pos_in_group

    src_pad = np.zeros(N_CORES * nb_tot * P, dtype=np.int16)
    slot_pad = np.full(N_CORES * nb_tot * P, 255.0, dtype=np.float32)
    src_pad[flat_pos] = src_s.astype(np.int16)
    slot_pad[flat_pos] = slot_of.astype(np.float32)
    src_pad = src_pad.reshape(N_CORES, nb_tot * P)
    slot_pad = slot_pad.reshape(N_CORES, nb_tot * P)

    feat_bf16 = feats.astype(BF16)
    wlt = W_l.T.astype(BF16).copy()
    wrt = W_r.T.astype(BF16).copy()
    bias1 = np.asarray(b_l, dtype=np.float32).reshape(1, HIDDEN).astype(BF16)
    ones1 = np.ones((1, HIDDEN), dtype=np.float32).astype(BF16)
    iota = np.broadcast_to(np.arange(P, dtype=np.float32), (P, P)).astype(BF16)

    in_maps = []
    for c in range(N_CORES):
        sp = src_pad[c]  # [nb_tot*128]
        sl = slot_pad[c]
        # idx wrap: position j -> partition j%16, column j//16; replicate x8.
        idx16 = sp.reshape(nb_tot * 8, 16).T  # [16, nb_tot*8]
        idx_full = np.tile(idx16, (8, 1))  # [128, nb_tot*8]
        # dslot: [p, col] = slot of edge (col*128 + p) within its tile block
        ds = sl.reshape(nb_tot, P).T.astype(BF16)  # [128, nb_tot]
        base = c * NODES_PER_CORE
        invrow = np.zeros(N_TILES * P, dtype=np.float32)
        invrow[:NODES_PER_CORE] = inv[base : base + NODES_PER_CORE]
        invb = np.broadcast_to(invrow, (P, N_TILES * P)).astype(BF16)
        xt = np.zeros((P, N_TILES * P), dtype=np.float32)
        xt[:, :NODES_PER_CORE] = feats[base : base + NODES_PER_CORE].T
        in_maps.append(
            {
                "feat": feat_bf16,
                "idx_all": np.ascontiguousarray(idx_full),
                "dslot": np.ascontiguousarray(ds),
                "invb": invb,
                "xt": xt.astype(BF16),
                "wlt": wlt,
                "wrt": wrt,
                "bias1": bias1,
                "ones1": ones1,
                "iota": np.ascontiguousarray(iota),
            }
        )
    return in_maps, nb_sched


def kernel(features, edge_index, W_l, b_l, W_r, _trace=False, _tmpdir=None):
    in_maps, nb_sched = _prepare_shards(features, edge_index, W_l, b_l, W_r)
    if nb_sched not in _compiled_cache:
        _compiled_cache[nb_sched] = _build_bass(nb_sched)
    nc = _compiled_cache[nb_sched]
    res = run_bass_kernel_spmd(
        nc,
        in_maps,
        core_ids=list(range(N_CORES)),
        trace=_trace,
        tmpdir=_tmpdir,
    )
    out = np.concatenate([res.results[c]["out"] for c in range(N_CORES)], axis=0)
    kernel._last_result = res
    return out.astype(np.float32)


# revision 48
# speedup vs baseline: 1.0248x; 1.0248x over previous
"""SAGEConv (mean aggregation) GNN message passing on 8 Trainium2 NeuronCores.

    out_i = lin_l(mean_{j:(j->i) in E} x_j) + lin_r(x_i)

Strategy (graph partitioning by destination node):
  - Host: shard dst nodes across 8 cores (2500 each). Per core, sort its
    incoming edges by dst, group into 20 tiles of 128 dst nodes. Per tile,
    edges are sorted by src (ascending HBM addresses improve DMA locality)
    and padded to nb_t blocks of 128 edges, where nb_t is the max block
    count across cores for tile slot t (keeps the program SPMD-identical).
  - Device (per core), per dst tile:
      * one large SWDGE dma_gather pulls all the tile's edge source rows
        (bf16, 256B each) from the replicated feature table in HBM into
        SBUF (amortizes the ~1us per-call SWDGE fixed cost).
      * DVE builds the scaled one-hot S[e,(b,d)] = (dslot_e,b == d) via
        is_equal against a materialized iota pattern; PE accumulates
        aggT[f,d] += sum_e M[e,f]*S[e,d] over the tile's blocks in PSUM.
      * aggT is scaled by 1/cnt during the PSUM->SBUF move (DVE mult).
      * PE applies W_l/W_r in bf16 (+ bias via a rank-1 matmul),
        accumulated in PSUM; scalar engine copies to SBUF; HWDGE DMA
        writes the 128-row output tile.
  - Host: concatenate the 8 per-core [2500, 128] outputs.
"""

import contextlib
import ctypes
import sys
import types

import ml_dtypes
import numpy as np

# ---------------------------------------------------------------------------
# NTFF profiling hook (lets run_bass_kernel_spmd(trace=True) work under axon;
# harmless if tracing is never requested).
# ---------------------------------------------------------------------------
_AXON_SO = "/opt/axon/libaxon_pjrt.so"


def _install_axon_ntff_hook():
    if "antenv.axon_hooks" in sys.modules:
        return
    try:
        lib = ctypes.CDLL(_AXON_SO)
        if not hasattr(lib, "axon_start_nrt_profile"):
            raise OSError("no profile symbols")
        lib.axon_start_nrt_profile.argtypes = [
            ctypes.POINTER(ctypes.c_int64),
            ctypes.c_size_t,
        ]
        lib.axon_start_nrt_profile.restype = ctypes.c_int64
        lib.axon_stop_nrt_profile.argtypes = [ctypes.c_char_p]
        lib.axon_stop_nrt_profile.restype = ctypes.c_int64

        @contextlib.contextmanager
        def _hook(output_dir, device_ids):
            import jax

            jax.devices()
            if device_ids:
                ids = (ctypes.c_int64 * len(device_ids))(*device_ids)
                rc = lib.axon_start_nrt_profile(ids, len(device_ids))
            else:
                rc = lib.axon_start_nrt_profile(None, 0)
            if rc != 0:
                raise RuntimeError(f"axon_start_nrt_profile rc={rc}")
            try:
                yield
            finally:
                n = lib.axon_stop_nrt_profile(str(output_dir).encode())
                print(f"ntff profile: {n} file(s) -> {output_dir}", file=sys.stderr)

        hook = _hook
    except OSError:
        hook = None

    mod = types.ModuleType("antenv.axon_hooks")
    mod._hook = hook
    mod.get_axon_ntff_profile_hook = lambda: mod._hook
    mod.set_axon_ntff_profile_hook = lambda h: setattr(mod, "_hook", h)
    sys.modules["antenv.axon_hooks"] = mod
    try:
        import antenv

        antenv.axon_hooks = mod
    except ImportError:
        pass


_install_axon_ntff_hook()

import concourse.bacc as bacc  # noqa: E402
import concourse.mybir as mybir  # noqa: E402
import concourse.tile as tile  # noqa: E402
from concourse.bass_utils import run_bass_kernel_spmd  # noqa: E402

# Problem shape (hardcoded per spec).
N_NODES = 20000
N_EDGES = 640000
HIDDEN = 128
N_CORES = 8
NODES_PER_CORE = N_NODES // N_CORES  # 2500
P = 128
N_TILES = -(-NODES_PER_CORE // P)  # 20 dst tiles per core (last has 68 rows)
LAST_ROWS = NODES_PER_CORE - (N_TILES - 1) * P  # 68

BF16 = ml_dtypes.bfloat16

_compiled_cache = {}


def _build_bass(nb_sched: tuple[int, ...]):
    """Per-core Bass program; nb_sched[t] = edge-block count for dst tile t."""
    nb_tot = sum(nb_sched)
    nc = bacc.Bacc(
        target_bir_lowering=False,
        num_swdge_queues=4,
        dynamic_dma_scratch_size=65536,
    )
    dt = mybir.dt

    feat = nc.dram_tensor("feat", [N_NODES, HIDDEN], dt.bfloat16, kind="ExternalInput")
    idx_all = nc.dram_tensor("idx_all", [P, nb_tot * 8], dt.int16, kind="ExternalInput")
    dslot = nc.dram_tensor("dslot", [P, nb_tot], dt.bfloat16, kind="ExternalInput")
    invb = nc.dram_tensor("invb", [P, N_TILES * P], dt.bfloat16, kind="ExternalInput")
    xt = nc.dram_tensor("xt", [P, N_TILES * P], dt.bfloat16, kind="ExternalInput")
    wlt = nc.dram_tensor("wlt", [P, HIDDEN], dt.bfloat16, kind="ExternalInput")
    wrt = nc.dram_tensor("wrt", [P, HIDDEN], dt.bfloat16, kind="ExternalInput")
    bias1 = nc.dram_tensor("bias1", [1, HIDDEN], dt.bfloat16, kind="ExternalInput")
    ones1 = nc.dram_tensor("ones1", [1, HIDDEN], dt.bfloat16, kind="ExternalInput")
    iota = nc.dram_tensor("iota", [P, P], dt.bfloat16, kind="ExternalInput")
    out = nc.dram_tensor("out", [NODES_PER_CORE, HIDDEN], dt.float32, kind="ExternalOutput")

    with tile.TileContext(nc) as tc:
        with (
            tc.tile_pool(name="const", bufs=1) as cpool,
            tc.tile_pool(name="meta", bufs=1) as mpool,
            tc.tile_pool(name="gath", bufs=5) as gpool,
            tc.tile_pool(name="sel", bufs=5) as spool,
            tc.tile_pool(name="aggs", bufs=2) as apool,
            tc.tile_pool(name="outs", bufs=2) as opool,
            tc.tile_pool(name="pagg", bufs=4, space="PSUM") as pagg_pool,
            tc.tile_pool(name="pout", bufs=2, space="PSUM") as pout_pool,
        ):
            # One-time loads.
            iota_t = cpool.tile([P, P], dt.bfloat16)
            wlt_t = cpool.tile([P, HIDDEN], dt.bfloat16, tag="wlt")
            wrt_t = cpool.tile([P, HIDDEN], dt.bfloat16, tag="wrt")
            bias1_t = cpool.tile([1, HIDDEN], dt.bfloat16, tag="bias1")
            ones1_t = cpool.tile([1, HIDDEN], dt.bfloat16, tag="ones1")
            xt_t = cpool.tile([P, N_TILES * P], dt.bfloat16, tag="xt")
            idx_t = mpool.tile([P, nb_tot * 8], dt.int16, tag="idx")
            dslot_t = mpool.tile([P, nb_tot], dt.bfloat16, tag="dslot")
            invb_t = mpool.tile([P, N_TILES * P], dt.bfloat16, tag="invb")
            nc.sync.dma_start(idx_t[:], idx_all[:])
            nc.sync.dma_start(dslot_t[:], dslot[:])
            nc.sync.dma_start(iota_t[:], iota[:])
            nc.sync.dma_start(wlt_t[:], wlt[:])
            nc.sync.dma_start(wrt_t[:], wrt[:])
            nc.sync.dma_start(bias1_t[:], bias1[:])
            nc.sync.dma_start(ones1_t[:], ones1[:])
            nc.sync.dma_start(xt_t[:], xt[:])
            nc.sync.dma_start(invb_t[:], invb[:])

            off = 0
            _qn = [0]
            for t in range(N_TILES):
                nb = nb_sched[t]
                g = gpool.tile([P, nb, HIDDEN], dt.bfloat16, tag="g")
                # Split each tile's gather across the 4 SWDGE queues: the four
                # Q7 pairs generate in parallel and the SDMA drains (the
                # per-tile wall) overlap.
                splits = [(j * nb) // 4 for j in range(5)]
                for j in range(4):
                    lo, hi = splits[j], splits[j + 1]
                    if hi == lo:
                        continue
                    nc.gpsimd.dma_gather(
                        g[:, lo:hi, :],
                        feat[:, :],
                        idx_t[:, (off + lo) * 8 : (off + hi) * 8],
                        num_idxs=(hi - lo) * P,
                        num_idxs_reg=(hi - lo) * P,
                        elem_size=HIDDEN,
                        queue_num=j,
                        single_packet=False,
                    )
                # One-hot S[e,(b,d)] = (dslot[e,b] == d), built in two steps so
                # the DVE compare hits the 2x perf tier (needs stride-1 last
                # dims on all operands): scalar engine replicates dslot along
                # d in-place, then DVE compares against the iota pattern.
                s = spool.tile([P, nb, P], dt.bfloat16, tag="s")
                nc.scalar.copy(
                    s[:],
                    dslot_t[:, off : off + nb][:, :, None].to_broadcast([P, nb, P]),
                )
                nc.vector.tensor_tensor(
                    s[:],
                    iota_t[:, None, :].to_broadcast([P, nb, P]),
                    s[:],
                    op=mybir.AluOpType.is_equal,
                )
                pa = pagg_pool.tile([P, P], dt.float32, tag="pa")
                for b in range(nb):
                    nc.tensor.matmul(
                        pa[:],
                        lhsT=g[:, b, :],
                        rhs=s[:, b, :],
                        start=(b == 0),
                        stop=(b == nb - 1),
                    )
                # mean: aggT = psum * (1/cnt[d]) during PSUM -> SBUF move.
                at = apool.tile([P, P], dt.bfloat16, tag="at")
                nc.vector.tensor_tensor(
                    at[:], pa[:], invb_t[:, t * P : (t + 1) * P], op=mybir.AluOpType.mult
                )
                po = pout_pool.tile([P, P], dt.float32, tag="po")
                nc.tensor.matmul(po[:], lhsT=ones1_t[:], rhs=bias1_t[:], start=True, stop=False)
                nc.tensor.matmul(po[:], lhsT=at[:], rhs=wlt_t[:], start=False, stop=False)
                nc.tensor.matmul(
                    po[:],
                    lhsT=xt_t[:, t * P : (t + 1) * P],
                    rhs=wrt_t[:],
                    start=False,
                    stop=True,
                )
                ob = opool.tile([P, P], dt.float32, tag="ob")
                nc.scalar.copy(ob[:], po[:])
                rows = LAST_ROWS if t == N_TILES - 1 else P
                nc.sync.dma_start(out[t * P : t * P + rows, :], ob[:rows, :])
                off += nb
    nc.compile()
    return nc


def _prepare_shards(features, edge_index, W_l, b_l, W_r):
    """Host-side graph partitioning -> per-core input maps + nb schedule."""
    src = np.asarray(edge_index[0], dtype=np.int64)
    dst = np.asarray(edge_index[1], dtype=np.int64)
    feats = np.asarray(features, dtype=np.float32)

    cnt = np.bincount(dst, minlength=N_NODES).astype(np.float32)
    inv = (1.0 / np.maximum(cnt, 1.0)).astype(np.float32)

    # Sort edges by (dst tile, src): group per (core, tile), ascending src
    # within each group for better HBM read locality during the gather.
    core_of_all = dst // NODES_PER_CORE
    off_all = dst - core_of_all * NODES_PER_CORE
    tile_of_all = off_all // P
    flat_ct_all = core_of_all * N_TILES + tile_of_all
    order = np.lexsort((src, flat_ct_all))
    src_s = src[order]
    dst_s = dst[order]
    flat_ct = flat_ct_all[order]
    slot_of = (dst_s % NODES_PER_CORE) % P

    # Edge counts per (core, tile); shared per-tile block schedule (max
    # across cores so the SPMD program is identical on every core).
    n_ct = N_CORES * N_TILES
    ct_cnt = np.bincount(flat_ct, minlength=n_ct).reshape(N_CORES, N_TILES)
    nb_sched = tuple(int(x) for x in np.maximum(-(-ct_cnt.max(axis=0) // P), 1))
    nb_tot = sum(nb_sched)
    tile_off = np.concatenate(([0], np.cumsum(nb_sched))).astype(np.int64)

    # Scatter each group's edges into its padded [nb_t*128] slot range.
    # Each tile's gather is issued as 4 splits (block ranges [(j*nb)//4]);
    # spread the group's real edges across the splits proportionally so
    # padding (-1 indices, dropped by the Q7) sits at each split's tail.
    starts = np.zeros(n_ct + 1, dtype=np.int64)
    np.cumsum(ct_cnt.reshape(-1), out=starts[1:])
    pos_in_group = np.arange(dst_s.shape[0]) - starts[flat_ct]
    core_idx = flat_ct // N_TILES
    tile_idx = flat_ct % N_TILES

    # flat position within the per-core padded edge array of length nb_tot*128
    flat_pos = (core_idx * nb_tot + tile_off[tile_idx]) * P + pos_in_group

    src_pad = np.zeros(N_CORES * nb_tot * P, dtype=np.int16)
    slot_pad = np.full(N_CORES * nb_tot * P, 255.0, dtype=np.float32)
    src_pad[flat_pos] = src_s.astype(np.int16)
    slot_pad[flat_pos] = slot_of.astype(np.float32)
    src_pad = src_pad.reshape(N_CORES, nb_tot * P)
    slot_pad = slot_pad.reshape(N_CORES, nb_tot * P)

    feat_bf16 = feats.astype(BF16)
    wlt = W_l.T.astype(BF16).copy()
    wrt = W_r.T.astype(BF16).copy()
    bias1 = np.asarray(b_l, dtype=np.float32).reshape(1, HIDDEN).astype(BF16)
    ones1 = np.ones((1, HIDDEN), dtype=np.float32).astype(BF16)
    iota = np.broadcast_to(np.arange(P, dtype=np.float32), (P, P)).astype(BF16)

    in_maps = []
    for c in range(N_CORES):
        sp = src_pad[c]  # [nb_tot*128]
        sl = slot_pad[c]
        # idx wrap: position j -> partition j%16, column j//16; replicate x8.
        idx16 = sp.reshape(nb_tot * 8, 16).T  # [16, nb_tot*8]
        idx_full = np.tile(idx16, (8, 1))  # [128, nb_tot*8]
        # dslot: [p, col] = slot of edge (col*128 + p) within its tile block
        ds = sl.reshape(nb_tot, P).T.astype(BF16)  # [128, nb_tot]
        base = c * NODES_PER_CORE
        invrow = np.zeros(N_TILES * P, dtype=np.float32)
        invrow[:NODES_PER_CORE] = inv[base : base + NODES_PER_CORE]
        invb = np.broadcast_to(invrow, (P, N_TILES * P)).astype(BF16)
        xt = np.zeros((P, N_TILES * P), dtype=np.float32)
        xt[:, :NODES_PER_CORE] = feats[base : base + NODES_PER_CORE].T
        in_maps.append(
            {
                "feat": feat_bf16,
                "idx_all": np.ascontiguousarray(idx_full),
                "dslot": np.ascontiguousarray(ds),
                "invb": invb,
                "xt": xt.astype(BF16),
                "wlt": wlt,
                "wrt": wrt,
                "bias1": bias1,
                "ones1": ones1,
                "iota": np.ascontiguousarray(iota),
            }
        )
    return in_maps, nb_sched


def kernel(features, edge_index, W_l, b_l, W_r, _trace=False, _tmpdir=None):
    in_maps, nb_sched = _prepare_shards(features, edge_index, W_l, b_l, W_r)
    if nb_sched not in _compiled_cache:
        _compiled_cache[nb_sched] = _build_bass(nb_sched)
    nc = _compiled_cache[nb_sched]
    res = run_bass_kernel_spmd(
        nc,
        in_maps,
        core_ids=list(range(N_CORES)),
        trace=_trace,
        tmpdir=_tmpdir,
    )
    out = np.concatenate([res.results[c]["out"] for c in range(N_CORES)], axis=0)
    kernel._last_result = res
    return out.astype(np.float32)
